# revision 1
# baseline (speedup 1.0000x reference)
"""LocalFrameAttentionWithDiffuser on 8 TRN2 NeuronCores.

Sharding: head-parallel. Each core computes 2 of the 16 heads end-to-end
(QKV projection for its 128 hd-dims, chunked local attention, partial
output projection Y_c = O_c @ Wo[c-slice]); the host sums the 8 partial
Y tensors (bias bo is fed only to core 0 so the sum adds it once).

Shapes (hardcoded from the problem):
  x [1,16,256,1024] -> tokens T=4096, D=1024, H=16 heads, HD=64,
  chunks C=4 of L=1024 tokens; chunk i attends to chunks {i-1, i}
  (chunk 0 only to itself).

Device layout notes:
  - everything flows transposed: X^T [D, T] is a host-prepared input so
    projections produce Q^T/K^T [hd, T] directly (hd on partitions).
  - S^T = K^T.T @ Q^T per (chunk, head) with ctx on partitions, so the
    softmax sum over ctx is computed by appending a ones-column to V in
    the AV matmul (row 64 of the AV PSUM accumulates sum(exp(s))).
  - chunk 0's missing previous chunk is handled by simply not issuing
    those ctx tiles (exactly reproduces the -inf mask).
  - matmuls use float32r (full-rate fp32 path on the PE).
"""

import os
from contextlib import ExitStack

import numpy as np

import concourse.bass as bass
import concourse.tile as tile
from concourse import bacc, mybir
from concourse.bass_utils import run_bass_kernel_spmd

F32 = mybir.dt.float32
F32R = mybir.dt.float32r

B, F, N, D = 1, 16, 256, 1024
H, HD = 16, 64
CS = 4
C = F // CS            # 4 chunks
L = CS * N             # 1024 tokens per chunk
T = F * N              # 4096 tokens
NCORES = 8
HPC = H // NCORES      # 2 heads per core
HDB = HPC * HD         # 128 hd dims per core
SCALE = 1.0 / np.sqrt(HD)

TOK_TILE = 512         # moving-dim tile (fp32 max)
NDT = D // 128         # 8 contraction tiles for projections
NJT = T // TOK_TILE    # 8 token tiles
NCT = T // 128         # 32 ctx tiles of 128


def _r(ap):
    return ap.bitcast(F32R)


def build_kernel(nc, tc, outs, ins, ctx, phases=3):
    xt, wq, wk, wv, wo, bo, ident = (
        ins["xt"], ins["wq"], ins["wk"], ins["wv"], ins["wo"], ins["bo"],
        ins["ident"],
    )
    y = outs["y"]

    # persistent pools: bufs=1, every tile gets a distinct name (= its own slot)
    wpool = ctx.enter_context(tc.tile_pool(name="weights", bufs=1))
    qk_pool = ctx.enter_context(tc.tile_pool(name="qk", bufs=1))
    v_pool = ctx.enter_context(tc.tile_pool(name="v", bufs=1))
    ot_pool = ctx.enter_context(tc.tile_pool(name="ot", bufs=1))
    ybias_pool = ctx.enter_context(tc.tile_pool(name="ybias", bufs=1))
    # cycling pools: shared tag -> bufs slots
    xpool = ctx.enter_context(tc.tile_pool(name="xt", bufs=16))
    vstage_pool = ctx.enter_context(tc.tile_pool(name="vstage", bufs=2))
    a_pool = ctx.enter_context(tc.tile_pool(name="attn", bufs=12))
    sum_pool = ctx.enter_context(tc.tile_pool(name="sums", bufs=8))
    bc_pool = ctx.enter_context(tc.tile_pool(name="bcast", bufs=4))
    yout_pool = ctx.enter_context(tc.tile_pool(name="yout", bufs=6))
    ps_pool = ctx.enter_context(tc.tile_pool(name="ps", bufs=8, space="PSUM"))
    proj_ps = vtr_ps = s_ps = o_ps = y_ps = ps_pool

    # ---- persistent weights / constants (loaded at first use) ----
    wq_sb = [wpool.tile([128, HDB], F32R, name=f"wq{d}") for d in range(NDT)]
    wk_sb = [wpool.tile([128, HDB], F32R, name=f"wk{d}") for d in range(NDT)]
    wv_sb = [wpool.tile([128, HDB], F32R, name=f"wv{d}") for d in range(NDT)]
    wo_sb = wpool.tile([128, D], F32R, tag="wo")
    id_sb = wpool.tile([128, 128], F32, tag="id")
    ones_col = wpool.tile([128, 1], F32, tag="ones")
    nc.vector.memset(ones_col[:], 1.0)
    bo_bc = ybias_pool.tile([128, D], F32)

    # persistent activations
    qt_sb = qk_pool.tile([128, T], F32R, tag="qt")   # Q^T (2 heads stacked)
    kt_sb = qk_pool.tile([128, T], F32R, tag="kt")   # K^T
    ot_sb = ot_pool.tile([128, T], F32R)             # O^T normalized
    # V per ctx tile: [128 tok, 65] (64 hd + ones column), per head
    v_sb = [[v_pool.tile([128, HD + 1], F32R, name=f"v{h}_{ct}") for ct in range(NCT)]
            for h in range(HPC)]

    # ---- phase 1: projections (per 512-token tile) ----
    for j in range(NJT):
        tok = bass.ts(j, TOK_TILE)
        xt_t = [xpool.tile([128, TOK_TILE], F32R, tag="x", name=f"xt{j}_{d}") for d in range(NDT)]
        for d in range(NDT):
            nc.sync.dma_start(xt_t[d][:], xt[d * 128:(d + 1) * 128, tok].bitcast(F32R))
            if j == 0:
                nc.sync.dma_start(wq_sb[d][:], wq[d * 128:(d + 1) * 128, :].bitcast(F32R))
                nc.sync.dma_start(wk_sb[d][:], wk[d * 128:(d + 1) * 128, :].bitcast(F32R))
                nc.sync.dma_start(wv_sb[d][:], wv[d * 128:(d + 1) * 128, :].bitcast(F32R))
        if j == 0:
            nc.sync.dma_start(id_sb[:], ident[:, :])
        if j == 2:
            nc.sync.dma_start(wo_sb[:], wo[:, :].bitcast(F32R))
            nc.sync.dma_start(bo_bc[:], bo[0:1, :].broadcast_to([128, D]))
        q_ps = proj_ps.tile([128, TOK_TILE], F32, tag="ps", name=f"qps{j}")
        k_ps = proj_ps.tile([128, TOK_TILE], F32, tag="ps", name=f"kps{j}")
        vt_ps = proj_ps.tile([128, TOK_TILE], F32, tag="ps", name=f"vps{j}")
        for d in range(NDT):
            st, sp = d == 0, d == NDT - 1
            nc.tensor.matmul(q_ps[:], wq_sb[d][:], xt_t[d][:], start=st, stop=sp)
            nc.tensor.matmul(k_ps[:], wk_sb[d][:], xt_t[d][:], start=st, stop=sp)
            nc.tensor.matmul(vt_ps[:], wv_sb[d][:], xt_t[d][:], start=st, stop=sp)
        nc.vector.tensor_copy(qt_sb[:, tok], q_ps[:])
        nc.vector.tensor_copy(kt_sb[:, tok], k_ps[:])
        vt_stage = vstage_pool.tile([128, TOK_TILE], F32, tag="vs", name=f"vst{j}")
        nc.vector.tensor_copy(vt_stage[:], vt_ps[:])
        # transpose V^T -> V in 128x128 blocks; split the two heads
        for kblk in range(TOK_TILE // 128):
            ct = j * (TOK_TILE // 128) + kblk
            vtr = vtr_ps.tile([128, 128], F32, tag="ps", name=f"vtr{j}_{kblk}")
            nc.tensor.transpose(vtr[:], vt_stage[:, bass.ts(kblk, 128)], id_sb[:])
            for h in range(HPC):
                nc.vector.tensor_copy(v_sb[h][ct][:, 0:HD], vtr[:, h * HD:(h + 1) * HD])
                nc.gpsimd.tensor_copy(v_sb[h][ct][:, HD:HD + 1], ones_col[:])

    # ---- phase 2+3: attention per chunk, then its slice of the output proj ----
    if phases < 2:
        return
    for c in range(C):
        cts = list(range(max(0, 8 * (c - 1)), 8 * (c + 1)))  # ctx tiles (128 tok)
        for th in range(L // TOK_TILE):  # 2 query halves per chunk
            tok0 = c * L + th * TOK_TILE
            tok = bass.ds(tok0, TOK_TILE)
            for h in range(HPC):
                hr = slice(h * HD, (h + 1) * HD)
                o_acc = o_ps.tile([HD + 1, TOK_TILE], F32, tag="ps", name=f"ops{c}_{th}_{h}")
                for ci, ct in enumerate(cts):
                    s_t = s_ps.tile([128, TOK_TILE], F32, tag="ps", name=f"sps{c}_{th}_{h}_{ci}")
                    nc.tensor.matmul(
                        s_t[:], kt_sb[hr, bass.ts(ct, 128)], qt_sb[hr, tok],
                        start=True, stop=True,
                    )
                    a_t = a_pool.tile([128, TOK_TILE], F32R, tag="a", name=f"a{c}_{th}_{h}_{ci}")
                    nc.scalar.activation(
                        a_t[:], s_t[:], mybir.ActivationFunctionType.Exp, scale=SCALE
                    )
                    nc.tensor.matmul(
                        o_acc[:], v_sb[h][ct][:], a_t[:],
                        start=(ci == 0), stop=(ci == len(cts) - 1),
                    )
                # normalize: rows 0:64 / row 64
                s_sum = sum_pool.tile([1, TOK_TILE], F32, tag="s", name=f"ssum{c}_{th}_{h}")
                nc.vector.reciprocal(s_sum[:], o_acc[HD:HD + 1, :])
                r_bc = bc_pool.tile([HD, TOK_TILE], F32, tag="bc", name=f"bc{c}_{th}_{h}")
                nc.gpsimd.partition_broadcast(r_bc[:], s_sum[0:1, :])
                nc.vector.tensor_mul(ot_sb[hr, tok], o_acc[0:HD, :], r_bc[:])
            # output projection for this half-chunk's 4 token tiles
            for m in ([] if phases < 3 else range(8 * c + 4 * th, 8 * c + 4 * (th + 1))):
                for dh in range(D // TOK_TILE):
                    yp = y_ps.tile([128, TOK_TILE], F32, tag="ps", name=f"yps{m}_{dh}")
                    nc.tensor.matmul(
                        yp[:], ot_sb[:, bass.ts(m, 128)],
                        wo_sb[:, bass.ts(dh, TOK_TILE)],
                        start=True, stop=True,
                    )
                    y_sb = yout_pool.tile([128, TOK_TILE], F32, tag="yo", name=f"yo{m}_{dh}")
                    nc.vector.tensor_add(y_sb[:], yp[:], bo_bc[:, bass.ts(dh, TOK_TILE)])
                    nc.sync.dma_start(y[bass.ts(m, 128), bass.ts(dh, TOK_TILE)], y_sb[:])


_CACHE = {}


def _build(phases=3):
    if ("nc", phases) in _CACHE:
        return _CACHE[("nc", phases)]
    nc = bacc.Bacc(
        "TRN2",
        target_bir_lowering=False,
        debug=False,
        enable_asserts=False,
        num_devices=NCORES,
    )
    ins = {
        "xt": nc.dram_tensor("xt", [D, T], F32, kind="ExternalInput").ap(),
        "wq": nc.dram_tensor("wq", [D, HDB], F32, kind="ExternalInput").ap(),
        "wk": nc.dram_tensor("wk", [D, HDB], F32, kind="ExternalInput").ap(),
        "wv": nc.dram_tensor("wv", [D, HDB], F32, kind="ExternalInput").ap(),
        "wo": nc.dram_tensor("wo", [HDB, D], F32, kind="ExternalInput").ap(),
        "bo": nc.dram_tensor("bo", [1, D], F32, kind="ExternalInput").ap(),
        "ident": nc.dram_tensor("ident", [128, 128], F32, kind="ExternalInput").ap(),
    }
    outs = {"y": nc.dram_tensor("y", [T, D], F32, kind="ExternalOutput").ap()}
    with tile.TileContext(nc, trace_sim=False) as tc:
        with ExitStack() as kctx:
            build_kernel(nc, tc, outs, ins, kctx, phases=phases)
    nc.compile()
    _CACHE[("nc", phases)] = nc
    return nc


def make_in_maps(x, Wq, Wk, Wv, Wo, bo):
    xt = np.ascontiguousarray(
        np.asarray(x, dtype=np.float32).reshape(T, D).T
    )
    ident = np.eye(128, dtype=np.float32)
    bo = np.asarray(bo, dtype=np.float32).reshape(1, D)
    zeros_bo = np.zeros_like(bo)
    in_maps = []
    for core in range(NCORES):
        hs = slice(core * HDB, (core + 1) * HDB)
        in_maps.append({
            "xt": xt,
            "wq": np.ascontiguousarray(np.asarray(Wq, np.float32)[:, hs]),
            "wk": np.ascontiguousarray(np.asarray(Wk, np.float32)[:, hs]),
            "wv": np.ascontiguousarray(np.asarray(Wv, np.float32)[:, hs]),
            "wo": np.ascontiguousarray(np.asarray(Wo, np.float32)[hs, :]),
            "bo": bo if core == 0 else zeros_bo,
            "ident": ident,
        })
    return in_maps


def kernel(x, Wq, Wk, Wv, Wo, bo, _trace=False, _tmpdir=None):
    nc = _build()
    in_maps = make_in_maps(x, Wq, Wk, Wv, Wo, bo)
    res = run_bass_kernel_spmd(
        nc, in_maps, core_ids=list(range(NCORES)),
        trace=_trace, tmpdir=_tmpdir,
        **({"trace_cores": list(range(NCORES))} if _trace else {}),
    )
    if _trace:
        kernel.last_results = res
    y = np.zeros((T, D), dtype=np.float32)
    for r in res.results:
        y += r["y"]
    return y.reshape(B, F, N, D)



# revision 4
# speedup vs baseline: 1.3047x; 1.3047x over previous
"""LocalFrameAttentionWithDiffuser on 8 TRN2 NeuronCores.

Sharding: head-parallel. Each core computes 2 of the 16 heads end-to-end
(QKV projection for its 128 hd-dims, chunked local attention, partial
output projection Y_c = O_c @ Wo[c-slice]); the host sums the 8 partial
Y tensors in fp32 and adds the bias once.

Shapes (hardcoded from the problem):
  x [1,16,256,1024] -> tokens T=4096, D=1024, H=16 heads, HD=64,
  chunks C=4 of L=1024 tokens; chunk i attends to chunks {i-1, i}
  (chunk 0 only to itself).

Device layout notes:
  - all matmul operands are bf16 (x, weights converted on host; DMA and
    SBUF halve vs fp32; PE rate is the same as fp32r).
  - S^T = K^T.T @ Q^T per (chunk, head) with ctx on partitions; exp runs
    on the scalar engine over [128, 2, 512] PSUM pairs (two ctx tiles
    per instruction to amortize the fixed access latency) and the
    softmax sum comes from a ones-column appended to V in the AV matmul.
  - chunk 0's missing previous chunk is handled by not issuing those ctx
    tiles (exactly reproduces the -inf mask).
  - partial Y is written bf16 straight after the output projection (the
    bias is added host-side, once); the host accumulates in fp32.
  - projections of chunk c+1 are interleaved between the attention heads
    of chunk c so the scalar engine (the bottleneck) never starves.
"""

import os
from contextlib import ExitStack

import numpy as np
import ml_dtypes

import concourse.bass as bass
import concourse.tile as tile
from concourse import bacc, mybir
from concourse.bass_utils import run_bass_kernel_spmd

F32 = mybir.dt.float32
BF16 = mybir.dt.bfloat16

B, F, N, D = 1, 16, 256, 1024
H, HD = 16, 64
CS = 4
C = F // CS            # 4 chunks
L = CS * N             # 1024 tokens per chunk
T = F * N              # 4096 tokens
NCORES = 8
HPC = H // NCORES      # 2 heads per core
HDB = HPC * HD         # 128 hd dims per core
SCALE = 1.0 / np.sqrt(HD)

TOK_TILE = 512
NDT = D // 128         # 8 contraction tiles for projections
NCT = T // 128         # 32 ctx tiles of 128


def build_kernel(nc, tc, outs, ins, ctx):
    xt = [ins[f"xt{i}"] for i in range(NDT)]
    wq, wk, wv, wo, ident = ins["wq"], ins["wk"], ins["wv"], ins["wo"], ins["ident"]
    y = outs["y"]

    wpool = ctx.enter_context(tc.tile_pool(name="weights", bufs=1))
    act_pool = ctx.enter_context(tc.tile_pool(name="acts", bufs=1))
    vs_pool = ctx.enter_context(tc.tile_pool(name="vstage", bufs=2))
    a_pool = ctx.enter_context(tc.tile_pool(name="attn", bufs=4))
    sum_pool = ctx.enter_context(tc.tile_pool(name="sums", bufs=4))
    bc_pool = ctx.enter_context(tc.tile_pool(name="bcast", bufs=4))
    yout_pool = ctx.enter_context(tc.tile_pool(name="yout", bufs=4))
    ps_pool = ctx.enter_context(tc.tile_pool(name="ps", bufs=1, space="PSUM"))

    # ---- persistent weights / activations ----
    wq_sb = [wpool.tile([128, HDB], BF16, name=f"wq{i}") for i in range(NDT)]
    wk_sb = [wpool.tile([128, HDB], BF16, name=f"wk{i}") for i in range(NDT)]
    wv_sb = [wpool.tile([128, HDB], BF16, name=f"wv{i}") for i in range(NDT)]
    wo_sb = wpool.tile([128, D], BF16, tag="wo")
    id_sb = wpool.tile([128, 128], BF16, tag="id")
    xt_sb = [wpool.tile([128, T], BF16, name=f"xs{i}") for i in range(NDT)]

    qt_sb = act_pool.tile([128, T], BF16, tag="qt")   # Q^T (2 heads stacked)
    kt_sb = act_pool.tile([128, T], BF16, tag="kt")   # K^T
    ot_sb = act_pool.tile([128, T], BF16, tag="ot")   # normalized O^T
    # V per head: [128 ctx, ct, 64 hd + ones]
    v_sb = [act_pool.tile([128, NCT, HD + 1], BF16, name=f"v{h}")
            for h in range(HPC)]
    for h in range(HPC):
        nc.vector.memset(v_sb[h][:, :, HD:HD + 1], 1.0)

    for i in range(NDT):
        nc.sync.dma_start(wq_sb[i][:], wq[i * 128:(i + 1) * 128, :])
        nc.sync.dma_start(wk_sb[i][:], wk[i * 128:(i + 1) * 128, :])
        nc.sync.dma_start(wv_sb[i][:], wv[i * 128:(i + 1) * 128, :])
    nc.sync.dma_start(wo_sb[:], wo[:, :])
    nc.sync.dma_start(id_sb[:], ident[:, :])

    def dma_x(c):
        for i in range(NDT):
            nc.sync.dma_start(xt_sb[i][:, c * L:(c + 1) * L],
                              xt[i][:, c * L:(c + 1) * L])

    def proj(j):
        """Project token tile j (512 tokens): Q^T/K^T bf16 + V tiles."""
        tok = slice(j * TOK_TILE, (j + 1) * TOK_TILE)
        q_ps = ps_pool.tile([128, TOK_TILE], F32, tag="qk", name=f"qps{j}")
        for i in range(NDT):
            nc.tensor.matmul(q_ps[:], wq_sb[i][:], xt_sb[i][:, tok],
                             start=(i == 0), stop=(i == NDT - 1))
        nc.vector.tensor_copy(qt_sb[:, tok], q_ps[:])
        k_ps = ps_pool.tile([128, TOK_TILE], F32, tag="qk", name=f"kps{j}")
        for i in range(NDT):
            nc.tensor.matmul(k_ps[:], wk_sb[i][:], xt_sb[i][:, tok],
                             start=(i == 0), stop=(i == NDT - 1))
        nc.vector.tensor_copy(kt_sb[:, tok], k_ps[:])
        v_ps = ps_pool.tile([128, TOK_TILE], F32, tag="qk", name=f"vps{j}")
        for i in range(NDT):
            nc.tensor.matmul(v_ps[:], wv_sb[i][:], xt_sb[i][:, tok],
                             start=(i == 0), stop=(i == NDT - 1))
        vt_stage = vs_pool.tile([128, TOK_TILE], BF16, tag="vs", name=f"vst{j}")
        nc.vector.tensor_copy(vt_stage[:], v_ps[:])
        for m in range(4):
            ct = j * 4 + m
            vtr = ps_pool.tile([128, 128], BF16, tag="yv", name=f"vtr{j}_{m}")
            nc.tensor.transpose(vtr[:], vt_stage[:, m * 128:(m + 1) * 128],
                                id_sb[:])
            for h in range(HPC):
                nc.vector.tensor_copy(v_sb[h][:, ct, 0:HD],
                                      vtr[:, h * HD:(h + 1) * HD])

    def attn_head(c, th, h, tag):
        """Attention for (chunk c, token half th, head h) -> ot_sb."""
        tok = slice(c * L + th * TOK_TILE, c * L + (th + 1) * TOK_TILE)
        hr = slice(h * HD, (h + 1) * HD)
        ct0 = max(0, 8 * (c - 1))
        cts = list(range(ct0, 8 * (c + 1)))
        o_ps = ps_pool.tile([HD + 1, TOK_TILE], F32, tag=f"o{tag}",
                            name=f"ops{c}_{th}_{h}")
        npair = len(cts) // 2
        for gi in range(npair):
            s2 = ps_pool.tile([128, 2, TOK_TILE], F32, tag="s", bufs=2,
                              name=f"sps{c}_{th}_{h}_{gi}")
            for kk in range(2):
                ct = cts[2 * gi + kk]
                nc.tensor.matmul(s2[:, kk, :],
                                 kt_sb[hr, ct * 128:(ct + 1) * 128],
                                 qt_sb[hr, tok], start=True, stop=True)
            a_t = a_pool.tile([128, 2, TOK_TILE], BF16, tag="a",
                              name=f"a{c}_{th}_{h}_{gi}")
            nc.scalar.activation(a_t[:], s2[:],
                                 mybir.ActivationFunctionType.Exp,
                                 scale=float(SCALE))
            for kk in range(2):
                ct = cts[2 * gi + kk]
                nc.tensor.matmul(o_ps[:], v_sb[h][:, ct, :], a_t[:, kk, :],
                                 start=(gi == 0 and kk == 0),
                                 stop=(gi == npair - 1 and kk == 1))
        s_sum = sum_pool.tile([1, TOK_TILE], F32, tag="ss", name=f"ssum{c}_{th}_{h}")
        nc.vector.reciprocal(s_sum[:], o_ps[HD:HD + 1, :])
        r_bc = bc_pool.tile([HD, TOK_TILE], F32, tag="bc", name=f"bc{c}_{th}_{h}")
        nc.gpsimd.partition_broadcast(r_bc[:], s_sum[0:1, :])
        nc.vector.tensor_mul(ot_sb[hr, tok], o_ps[0:HD, :], r_bc[:])

    def yproj(c, th):
        """Output projection for the 4 token tiles of half-chunk (c, th)."""
        for mi in range(4):
            m = c * 8 + th * 4 + mi
            for dh in range(2):
                ds = slice(dh * TOK_TILE, (dh + 1) * TOK_TILE)
                y_ps = ps_pool.tile([128, TOK_TILE], F32, tag="yv",
                                    name=f"yps{m}_{dh}")
                nc.tensor.matmul(y_ps[:], ot_sb[:, m * 128:(m + 1) * 128],
                                 wo_sb[:, ds], start=True, stop=True)
                y_sb = yout_pool.tile([128, TOK_TILE], BF16, tag="yo",
                                      name=f"yo{m}_{dh}")
                nc.vector.tensor_copy(y_sb[:], y_ps[:])
                nc.sync.dma_start(y[m * 128:(m + 1) * 128, ds], y_sb[:])

    # ---- schedule ----
    dma_x(0)
    proj(0)
    proj(1)
    for c in range(C):
        if c + 1 < C:
            dma_x(c + 1)
        attn_head(c, 0, 0, 0)
        if c + 1 < C:
            proj(2 * (c + 1))
        attn_head(c, 0, 1, 1)
        if c + 1 < C:
            proj(2 * (c + 1) + 1)
        yproj(c, 0)
        attn_head(c, 1, 0, 0)
        attn_head(c, 1, 1, 1)
        yproj(c, 1)


_CACHE = {}


def _build():
    if "nc" in _CACHE:
        return _CACHE["nc"]
    nc = bacc.Bacc(
        "TRN2",
        target_bir_lowering=False,
        debug=False,
        enable_asserts=False,
        num_devices=NCORES,
    )
    ins = {}
    for i in range(NDT):
        ins[f"xt{i}"] = nc.dram_tensor(f"xt{i}", [128, T], BF16,
                                       kind="ExternalInput").ap()
    for nm in ("wq", "wk", "wv"):
        ins[nm] = nc.dram_tensor(nm, [D, HDB], BF16, kind="ExternalInput").ap()
    ins["wo"] = nc.dram_tensor("wo", [HDB, D], BF16, kind="ExternalInput").ap()
    ins["ident"] = nc.dram_tensor("ident", [128, 128], BF16,
                                  kind="ExternalInput").ap()
    outs = {"y": nc.dram_tensor("y", [T, D], BF16, kind="ExternalOutput").ap()}
    with tile.TileContext(nc, trace_sim=False) as tc:
        with ExitStack() as kctx:
            build_kernel(nc, tc, outs, ins, kctx)
    nc.compile()
    _CACHE["nc"] = nc
    return nc


def make_in_maps(x, Wq, Wk, Wv, Wo, bo):
    xv = np.asarray(x, np.float32).reshape(T, D).T          # [D, T]
    x16 = np.ascontiguousarray(xv).astype(ml_dtypes.bfloat16)
    ident = np.eye(128, dtype=np.float32).astype(ml_dtypes.bfloat16)
    in_maps = []
    for core in range(NCORES):
        hs = slice(core * HDB, (core + 1) * HDB)
        m = {f"xt{i}": x16[i * 128:(i + 1) * 128] for i in range(NDT)}
        m["wq"] = np.ascontiguousarray(
            np.asarray(Wq, np.float32)[:, hs]).astype(ml_dtypes.bfloat16)
        m["wk"] = np.ascontiguousarray(
            np.asarray(Wk, np.float32)[:, hs]).astype(ml_dtypes.bfloat16)
        m["wv"] = np.ascontiguousarray(
            np.asarray(Wv, np.float32)[:, hs]).astype(ml_dtypes.bfloat16)
        m["wo"] = np.ascontiguousarray(
            np.asarray(Wo, np.float32)[hs, :]).astype(ml_dtypes.bfloat16)
        m["ident"] = ident
        in_maps.append(m)
    return in_maps


def kernel(x, Wq, Wk, Wv, Wo, bo, _trace=False, _tmpdir=None):
    nc = _build()
    in_maps = make_in_maps(x, Wq, Wk, Wv, Wo, bo)
    res = run_bass_kernel_spmd(
        nc, in_maps, core_ids=list(range(NCORES)),
        trace=_trace, tmpdir=_tmpdir,
        **({"trace_cores": list(range(NCORES))} if _trace else {}),
    )
    if _trace:
        kernel.last_results = res
    y = np.zeros((T, D), dtype=np.float32)
    for r in res.results:
        y += np.asarray(r["y"], dtype=np.float32)
    y += np.asarray(bo, np.float32).reshape(1, D)
    return y.reshape(B, F, N, D)


# revision 7
# speedup vs baseline: 1.3581x; 1.0410x over previous
"""LocalFrameAttentionWithDiffuser on 8 TRN2 NeuronCores.

Sharding: head-parallel. Each core computes 2 of the 16 heads end-to-end
(QKV projection for its 128 hd-dims, chunked local attention, partial
output projection Y_c = O_c @ Wo[c-slice]); the host sums the 8 partial
Y tensors in fp32 and adds the bias once.

Shapes (hardcoded from the problem):
  x [1,16,256,1024] -> tokens T=4096, D=1024, H=16 heads, HD=64,
  chunks C=4 of L=1024 tokens; chunk i attends to chunks {i-1, i}
  (chunk 0 only to itself).

Device layout notes:
  - all matmul operands are bf16 (x, weights converted on host; DMA and
    SBUF halve vs fp32; PE rate is the same as fp32r).
  - S^T = K^T.T @ Q^T per (chunk, head) with ctx on partitions; exp runs
    on the scalar engine over [128, 2, 512] PSUM pairs (two ctx tiles
    per instruction to amortize the fixed access latency) and the
    softmax sum comes from a ones-column appended to V in the AV matmul.
  - chunk 0's missing previous chunk is handled by not issuing those ctx
    tiles (exactly reproduces the -inf mask).
  - partial Y is written bf16 straight after the output projection (the
    bias is added host-side, once); the host accumulates in fp32.
  - projections of chunk c+1 are interleaved between the attention heads
    of chunk c so the scalar engine (the bottleneck) never starves.
"""

import os
from contextlib import ExitStack

import numpy as np
import ml_dtypes

import concourse.bass as bass
import concourse.tile as tile
from concourse import bacc, mybir
from concourse.bass_utils import run_bass_kernel_spmd

F32 = mybir.dt.float32
BF16 = mybir.dt.bfloat16

B, F, N, D = 1, 16, 256, 1024
H, HD = 16, 64
CS = 4
C = F // CS            # 4 chunks
L = CS * N             # 1024 tokens per chunk
T = F * N              # 4096 tokens
NCORES = 8
HPC = H // NCORES      # 2 heads per core
HDB = HPC * HD         # 128 hd dims per core
SCALE = 1.0 / np.sqrt(HD)

TOK_TILE = 512
NDT = D // 128         # 8 contraction tiles for projections
NCT = T // 128         # 32 ctx tiles of 128


def build_kernel(nc, tc, outs, ins, ctx):
    xt = [ins[f"xt{i}"] for i in range(NDT)]
    wq, wk, wv, wo, ident = ins["wq"], ins["wk"], ins["wv"], ins["wo"], ins["ident"]
    y = outs["y"]

    wpool = ctx.enter_context(tc.tile_pool(name="weights", bufs=1))
    act_pool = ctx.enter_context(tc.tile_pool(name="acts", bufs=1))
    vs_pool = ctx.enter_context(tc.tile_pool(name="vstage", bufs=2))
    a_pool = ctx.enter_context(tc.tile_pool(name="attn", bufs=4))
    sum_pool = ctx.enter_context(tc.tile_pool(name="sums", bufs=4))
    bc_pool = ctx.enter_context(tc.tile_pool(name="bcast", bufs=4))
    yout_pool = ctx.enter_context(tc.tile_pool(name="yout", bufs=4))
    ps_pool = ctx.enter_context(tc.tile_pool(name="ps", bufs=1, space="PSUM"))

    # ---- persistent weights / activations ----
    wq_sb = wpool.tile([128, NDT, HDB], BF16, tag="wq")
    wk_sb = wpool.tile([128, NDT, HDB], BF16, tag="wk")
    wv_sb = wpool.tile([128, NDT, HDB], BF16, tag="wv")
    wo_sb = wpool.tile([128, D], BF16, tag="wo")
    id_sb = wpool.tile([128, 128], BF16, tag="id")
    xt_sb = [wpool.tile([128, T], BF16, name=f"xs{i}") for i in range(NDT)]

    qt_sb = act_pool.tile([128, T], BF16, tag="qt")   # Q^T (2 heads stacked)
    kt_sb = act_pool.tile([128, T], BF16, tag="kt")   # K^T
    ot_sb = act_pool.tile([128, T], BF16, tag="ot")   # normalized O^T
    # V per head: [128 ctx, ct, 64 hd + ones]
    v_sb = [act_pool.tile([128, NCT, HD + 1], BF16, name=f"v{h}")
            for h in range(HPC)]
    for h in range(HPC):
        nc.vector.memset(v_sb[h][:, :, HD:HD + 1], 1.0)

    def dma_x(c):
        for i in range(NDT):
            nc.sync.dma_start(xt_sb[i][:, c * L:(c + 1) * L],
                              xt[i][:, c * L:(c + 1) * L])

    dma_x(0)
    nc.sync.dma_start(wq_sb[:], wq.rearrange("(i p) m -> p i m", i=NDT))
    nc.sync.dma_start(wk_sb[:], wk.rearrange("(i p) m -> p i m", i=NDT))
    nc.sync.dma_start(wv_sb[:], wv.rearrange("(i p) m -> p i m", i=NDT))
    nc.sync.dma_start(wo_sb[:], wo[:, :])
    nc.sync.dma_start(id_sb[:], ident[:, :])

    def proj_qk(j, w_sb, dst, tag):
        tok = slice(j * TOK_TILE, (j + 1) * TOK_TILE)
        p_ps = ps_pool.tile([128, TOK_TILE], F32, tag=tag, name=f"p{j}_{tag}")
        for i in range(NDT):
            nc.tensor.matmul(p_ps[:], w_sb[:, i, :], xt_sb[i][:, tok],
                             start=(i == 0), stop=(i == NDT - 1))
        nc.vector.tensor_copy(dst[:, tok], p_ps[:])

    def proj_v(j, tag):
        tok = slice(j * TOK_TILE, (j + 1) * TOK_TILE)
        v_ps = ps_pool.tile([128, TOK_TILE], F32, tag=tag, name=f"vps{j}")
        for i in range(NDT):
            nc.tensor.matmul(v_ps[:], wv_sb[:, i, :], xt_sb[i][:, tok],
                             start=(i == 0), stop=(i == NDT - 1))
        vt_stage = vs_pool.tile([128, TOK_TILE], BF16, tag="vs", name=f"vst{j}")
        nc.vector.tensor_copy(vt_stage[:], v_ps[:])
        for m in range(4):
            ct = j * 4 + m
            vtr = ps_pool.tile([128, 128], BF16, tag="yv", name=f"vtr{j}_{m}")
            nc.tensor.transpose(vtr[:], vt_stage[:, m * 128:(m + 1) * 128],
                                id_sb[:])
            for h in range(HPC):
                nc.vector.tensor_copy(v_sb[h][:, ct, 0:HD],
                                      vtr[:, h * HD:(h + 1) * HD])

    def attn_head(c, th, h, tag):
        """Attention for (chunk c, token half th, head h) -> ot_sb."""
        tok = slice(c * L + th * TOK_TILE, c * L + (th + 1) * TOK_TILE)
        hr = slice(h * HD, (h + 1) * HD)
        ct0 = max(0, 8 * (c - 1))
        cts = list(range(ct0, 8 * (c + 1)))
        o_ps = ps_pool.tile([HD + 1, TOK_TILE], F32, tag=f"o{tag}",
                            name=f"ops{c}_{th}_{h}")
        npair = len(cts) // 2
        for gi in range(npair):
            s2 = ps_pool.tile([128, 2, TOK_TILE], F32, tag="s", bufs=2,
                              name=f"sps{c}_{th}_{h}_{gi}")
            for kk in range(2):
                ct = cts[2 * gi + kk]
                nc.tensor.matmul(s2[:, kk, :],
                                 kt_sb[hr, ct * 128:(ct + 1) * 128],
                                 qt_sb[hr, tok], start=True, stop=True)
            a_t = a_pool.tile([128, 2, TOK_TILE], BF16, tag="a",
                              name=f"a{c}_{th}_{h}_{gi}")
            nc.scalar.activation(a_t[:], s2[:],
                                 mybir.ActivationFunctionType.Exp,
                                 scale=float(SCALE))
            for kk in range(2):
                ct = cts[2 * gi + kk]
                nc.tensor.matmul(o_ps[:], v_sb[h][:, ct, :], a_t[:, kk, :],
                                 start=(gi == 0 and kk == 0),
                                 stop=(gi == npair - 1 and kk == 1))
        s_sum = sum_pool.tile([1, TOK_TILE], F32, tag="ss", name=f"ssum{c}_{th}_{h}")
        nc.vector.reciprocal(s_sum[:], o_ps[HD:HD + 1, :])
        r_bc = bc_pool.tile([HD, TOK_TILE], F32, tag="bc", name=f"bc{c}_{th}_{h}")
        nc.gpsimd.partition_broadcast(r_bc[:], s_sum[0:1, :])
        nc.vector.tensor_mul(ot_sb[hr, tok], o_ps[0:HD, :], r_bc[:])

    def yproj(c, th):
        """Output projection for the 4 token tiles of half-chunk (c, th)."""
        m0 = c * 8 + th * 4
        for dh in range(2):
            ds = slice(dh * TOK_TILE, (dh + 1) * TOK_TILE)
            y_sb = yout_pool.tile([128, 4, TOK_TILE], BF16, tag="yo",
                                  name=f"yo{c}_{th}_{dh}")
            for mi in range(4):
                m = m0 + mi
                y_ps = ps_pool.tile([128, TOK_TILE], F32, tag="yv",
                                    name=f"yps{m}_{dh}")
                nc.tensor.matmul(y_ps[:], ot_sb[:, m * 128:(m + 1) * 128],
                                 wo_sb[:, ds], start=True, stop=True)
                nc.vector.tensor_copy(y_sb[:, mi, :], y_ps[:])
            nc.sync.dma_start(
                y[m0 * 128:(m0 + 4) * 128, ds]
                .rearrange("(m p) t -> p m t", m=4), y_sb[:])

    # ---- schedule ----
    # prologue: chunk-0 projections, pipelined through 3 psum slots
    for j, tg in ((0, "qk"), (1, "o0")):
        proj_qk(j, wq_sb, qt_sb, tg)
    for j, tg in ((0, "o1"), (1, "qk")):
        proj_qk(j, wk_sb, kt_sb, tg)
    proj_v(0, "o0")
    proj_v(1, "o1")
    for c in range(C):
        if c + 1 < C:
            dma_x(c + 1)
        j0, j1 = 2 * (c + 1), 2 * (c + 1) + 1
        attn_head(c, 0, 0, 0)
        if c + 1 < C:
            proj_qk(j0, wq_sb, qt_sb, "qk")
        attn_head(c, 0, 1, 1)
        if c + 1 < C:
            proj_qk(j1, wq_sb, qt_sb, "qk")
        yproj(c, 0)
        if c + 1 < C:
            proj_qk(j0, wk_sb, kt_sb, "qk")
        attn_head(c, 1, 0, 0)
        if c + 1 < C:
            proj_qk(j1, wk_sb, kt_sb, "qk")
        attn_head(c, 1, 1, 1)
        if c + 1 < C:
            proj_v(j0, "qk")
        yproj(c, 1)
        if c + 1 < C:
            proj_v(j1, "qk")


_CACHE = {}


def _build():
    if "nc" in _CACHE:
        return _CACHE["nc"]
    nc = bacc.Bacc(
        "TRN2",
        target_bir_lowering=False,
        debug=False,
        enable_asserts=False,
        num_devices=NCORES,
    )
    ins = {}
    for i in range(NDT):
        ins[f"xt{i}"] = nc.dram_tensor(f"xt{i}", [128, T], BF16,
                                       kind="ExternalInput").ap()
    for nm in ("wq", "wk", "wv"):
        ins[nm] = nc.dram_tensor(nm, [D, HDB], BF16, kind="ExternalInput").ap()
    ins["wo"] = nc.dram_tensor("wo", [HDB, D], BF16, kind="ExternalInput").ap()
    ins["ident"] = nc.dram_tensor("ident", [128, 128], BF16,
                                  kind="ExternalInput").ap()
    outs = {"y": nc.dram_tensor("y", [T, D], BF16, kind="ExternalOutput").ap()}
    with tile.TileContext(nc, trace_sim=False) as tc:
        with ExitStack() as kctx:
            build_kernel(nc, tc, outs, ins, kctx)
    nc.compile()
    _CACHE["nc"] = nc
    return nc


def make_in_maps(x, Wq, Wk, Wv, Wo, bo):
    xv = np.asarray(x, np.float32).reshape(T, D).T          # [D, T]
    x16 = np.ascontiguousarray(xv).astype(ml_dtypes.bfloat16)
    ident = np.eye(128, dtype=np.float32).astype(ml_dtypes.bfloat16)
    in_maps = []
    for core in range(NCORES):
        hs = slice(core * HDB, (core + 1) * HDB)
        m = {f"xt{i}": x16[i * 128:(i + 1) * 128] for i in range(NDT)}
        m["wq"] = np.ascontiguousarray(
            np.asarray(Wq, np.float32)[:, hs]).astype(ml_dtypes.bfloat16)
        m["wk"] = np.ascontiguousarray(
            np.asarray(Wk, np.float32)[:, hs]).astype(ml_dtypes.bfloat16)
        m["wv"] = np.ascontiguousarray(
            np.asarray(Wv, np.float32)[:, hs]).astype(ml_dtypes.bfloat16)
        m["wo"] = np.ascontiguousarray(
            np.asarray(Wo, np.float32)[hs, :]).astype(ml_dtypes.bfloat16)
        m["ident"] = ident
        in_maps.append(m)
    return in_maps


def kernel(x, Wq, Wk, Wv, Wo, bo, _trace=False, _tmpdir=None):
    nc = _build()
    in_maps = make_in_maps(x, Wq, Wk, Wv, Wo, bo)
    res = run_bass_kernel_spmd(
        nc, in_maps, core_ids=list(range(NCORES)),
        trace=_trace, tmpdir=_tmpdir,
        **({"trace_cores": list(range(NCORES))} if _trace else {}),
    )
    if _trace:
        kernel.last_results = res
    y = np.zeros((T, D), dtype=np.float32)
    for r in res.results:
        y += np.asarray(r["y"], dtype=np.float32)
    y += np.asarray(bo, np.float32).reshape(1, D)
    return y.reshape(B, F, N, D)


# revision 9
# speedup vs baseline: 1.3607x; 1.0019x over previous
"""LocalFrameAttentionWithDiffuser on 8 TRN2 NeuronCores.

Sharding: head-parallel. Each core computes 2 of the 16 heads end-to-end
(QKV projection for its 128 hd-dims, chunked local attention, partial
output projection Y_c = O_c @ Wo[c-slice]); the host sums the 8 partial
Y tensors in fp32 and adds the bias once.

Shapes (hardcoded from the problem):
  x [1,16,256,1024] -> tokens T=4096, D=1024, H=16 heads, HD=64,
  chunks C=4 of L=1024 tokens; chunk i attends to chunks {i-1, i}
  (chunk 0 only to itself).

Device layout notes:
  - all matmul operands are bf16 (x, weights converted on host; DMA and
    SBUF halve vs fp32; PE rate is the same as fp32r).
  - S^T = K^T.T @ Q^T per (chunk, head) with ctx on partitions; exp runs
    on the scalar engine over [128, 2, 512] PSUM pairs (two ctx tiles
    per instruction to amortize the fixed access latency) and the
    softmax sum comes from a ones-column appended to V in the AV matmul.
  - chunk 0's missing previous chunk is handled by not issuing those ctx
    tiles (exactly reproduces the -inf mask).
  - partial Y is written bf16 straight after the output projection (the
    bias is added host-side, once); the host accumulates in fp32.
  - projections of chunk c+1 are interleaved between the attention heads
    of chunk c so the scalar engine (the bottleneck) never starves.
"""

import os
from contextlib import ExitStack

import numpy as np
import ml_dtypes

import concourse.bass as bass
import concourse.tile as tile
from concourse import bacc, mybir
from concourse.bass_utils import run_bass_kernel_spmd

F32 = mybir.dt.float32
BF16 = mybir.dt.bfloat16

B, F, N, D = 1, 16, 256, 1024
H, HD = 16, 64
CS = 4
C = F // CS            # 4 chunks
L = CS * N             # 1024 tokens per chunk
T = F * N              # 4096 tokens
NCORES = 8
HPC = H // NCORES      # 2 heads per core
HDB = HPC * HD         # 128 hd dims per core
SCALE = 1.0 / np.sqrt(HD)

TOK_TILE = 512
NDT = D // 128         # 8 contraction tiles for projections
NCT = T // 128         # 32 ctx tiles of 128


def build_kernel(nc, tc, outs, ins, ctx):
    xt = [ins[f"xt{i}"] for i in range(NDT)]
    wq, wk, wv, wo, ident = ins["wq"], ins["wk"], ins["wv"], ins["wo"], ins["ident"]
    y = outs["y"]

    wpool = ctx.enter_context(tc.tile_pool(name="weights", bufs=1))
    act_pool = ctx.enter_context(tc.tile_pool(name="acts", bufs=1))
    vs_pool = ctx.enter_context(tc.tile_pool(name="vstage", bufs=2))
    a_pool = ctx.enter_context(tc.tile_pool(name="attn", bufs=4))
    sum_pool = ctx.enter_context(tc.tile_pool(name="sums", bufs=4))
    bc_pool = ctx.enter_context(tc.tile_pool(name="bcast", bufs=4))
    yout_pool = ctx.enter_context(tc.tile_pool(name="yout", bufs=4))
    ps_pool = ctx.enter_context(tc.tile_pool(name="ps", bufs=1, space="PSUM"))

    # ---- persistent weights / activations ----
    wq_sb = wpool.tile([128, NDT, HDB], BF16, tag="wq")
    wk_sb = wpool.tile([128, NDT, HDB], BF16, tag="wk")
    wv_sb = wpool.tile([128, NDT, HDB], BF16, tag="wv")
    wo_sb = wpool.tile([128, D], BF16, tag="wo")
    id_sb = wpool.tile([128, 128], BF16, tag="id")
    xt_sb = [wpool.tile([128, T], BF16, name=f"xs{i}") for i in range(NDT)]

    qt_sb = act_pool.tile([128, T], BF16, tag="qt")   # Q^T (2 heads stacked)
    kt_sb = act_pool.tile([128, T], BF16, tag="kt")   # K^T
    ot_sb = act_pool.tile([128, T], BF16, tag="ot")   # normalized O^T
    # V per head: [128 ctx, ct, 64 hd + ones]
    v_sb = [act_pool.tile([128, NCT, HD + 1], BF16, name=f"v{h}")
            for h in range(HPC)]
    for h in range(HPC):
        nc.vector.memset(v_sb[h][:, :, HD:HD + 1], 1.0)

    def dma_x(c):
        for i in range(NDT):
            nc.sync.dma_start(xt_sb[i][:, c * L:(c + 1) * L],
                              xt[i][:, c * L:(c + 1) * L])

    dma_x(0)
    nc.sync.dma_start(wq_sb[:], wq.rearrange("(i p) m -> p i m", i=NDT))
    nc.sync.dma_start(wk_sb[:], wk.rearrange("(i p) m -> p i m", i=NDT))
    nc.sync.dma_start(wv_sb[:], wv.rearrange("(i p) m -> p i m", i=NDT))
    nc.sync.dma_start(wo_sb[:], wo[:, :])
    nc.sync.dma_start(id_sb[:], ident[:, :])

    def proj_qk(j, w_sb, dst, tag):
        tok = slice(j * TOK_TILE, (j + 1) * TOK_TILE)
        p_ps = ps_pool.tile([128, TOK_TILE], F32, tag=tag, name=f"p{j}_{tag}")
        for i in range(NDT):
            nc.tensor.matmul(p_ps[:], w_sb[:, i, :], xt_sb[i][:, tok],
                             start=(i == 0), stop=(i == NDT - 1))
        nc.vector.tensor_copy(dst[:, tok], p_ps[:])

    def proj_v(j, tag):
        tok = slice(j * TOK_TILE, (j + 1) * TOK_TILE)
        v_ps = ps_pool.tile([128, TOK_TILE], F32, tag=tag, name=f"vps{j}")
        for i in range(NDT):
            nc.tensor.matmul(v_ps[:], wv_sb[:, i, :], xt_sb[i][:, tok],
                             start=(i == 0), stop=(i == NDT - 1))
        vt_stage = vs_pool.tile([128, TOK_TILE], BF16, tag="vs", name=f"vst{j}")
        nc.vector.tensor_copy(vt_stage[:], v_ps[:])
        for m in range(4):
            ct = j * 4 + m
            vtr = ps_pool.tile([128, 128], BF16, tag="yv", name=f"vtr{j}_{m}")
            nc.tensor.transpose(vtr[:], vt_stage[:, m * 128:(m + 1) * 128],
                                id_sb[:])
            for h in range(HPC):
                nc.vector.tensor_copy(v_sb[h][:, ct, 0:HD],
                                      vtr[:, h * HD:(h + 1) * HD])

    def attn_head(c, th, h, tag):
        """Attention for (chunk c, token half th, head h) -> ot_sb."""
        tok = slice(c * L + th * TOK_TILE, c * L + (th + 1) * TOK_TILE)
        hr = slice(h * HD, (h + 1) * HD)
        ct0 = max(0, 8 * (c - 1))
        cts = list(range(ct0, 8 * (c + 1)))
        o_ps = ps_pool.tile([HD + 1, TOK_TILE], F32, tag=f"o{tag}",
                            name=f"ops{c}_{th}_{h}")
        npair = len(cts) // 2
        for gi in range(npair):
            s2 = ps_pool.tile([128, 2, TOK_TILE], F32, tag="s", bufs=2,
                              name=f"sps{c}_{th}_{h}_{gi}")
            for kk in range(2):
                ct = cts[2 * gi + kk]
                nc.tensor.matmul(s2[:, kk, :],
                                 kt_sb[hr, ct * 128:(ct + 1) * 128],
                                 qt_sb[hr, tok], start=True, stop=True)
            a_t = a_pool.tile([128, 2, TOK_TILE], BF16, tag="a",
                              name=f"a{c}_{th}_{h}_{gi}")
            nc.scalar.activation(a_t[:], s2[:],
                                 mybir.ActivationFunctionType.Exp,
                                 scale=float(SCALE))
            for kk in range(2):
                ct = cts[2 * gi + kk]
                nc.tensor.matmul(o_ps[:], v_sb[h][:, ct, :], a_t[:, kk, :],
                                 start=(gi == 0 and kk == 0),
                                 stop=(gi == npair - 1 and kk == 1))
        s_sum = sum_pool.tile([1, TOK_TILE], F32, tag="ss", name=f"ssum{c}_{th}_{h}")
        nc.vector.reciprocal(s_sum[:], o_ps[HD:HD + 1, :])
        r_bc = bc_pool.tile([HD, TOK_TILE], F32, tag="bc", name=f"bc{c}_{th}_{h}")
        nc.gpsimd.partition_broadcast(r_bc[:], s_sum[0:1, :])
        nc.vector.tensor_mul(ot_sb[hr, tok], o_ps[0:HD, :], r_bc[:])

    def yproj(c, th):
        """Output projection for the 4 token tiles of half-chunk (c, th)."""
        m0 = c * 8 + th * 4
        for dh in range(2):
            ds = slice(dh * TOK_TILE, (dh + 1) * TOK_TILE)
            y_sb = yout_pool.tile([128, 4, TOK_TILE], BF16, tag="yo",
                                  name=f"yo{c}_{th}_{dh}")
            for mi in range(4):
                m = m0 + mi
                y_ps = ps_pool.tile([128, TOK_TILE], F32, tag="yv",
                                    name=f"yps{m}_{dh}")
                nc.tensor.matmul(y_ps[:], ot_sb[:, m * 128:(m + 1) * 128],
                                 wo_sb[:, ds], start=True, stop=True)
                nc.vector.tensor_copy(y_sb[:, mi, :], y_ps[:])
            nc.sync.dma_start(
                y[m0 * 128:(m0 + 4) * 128, ds]
                .rearrange("(m p) t -> p m t", m=4), y_sb[:])

    # ---- schedule ----
    # prologue: chunk-0 projections, pipelined through 3 psum slots
    for j, tg in ((0, "qk"), (1, "o0")):
        proj_qk(j, wq_sb, qt_sb, tg)
    for j, tg in ((0, "o1"), (1, "qk")):
        proj_qk(j, wk_sb, kt_sb, tg)
    proj_v(0, "o0")
    proj_v(1, "o1")
    for c in range(C):
        if c + 1 < C:
            dma_x(c + 1)
        j0, j1 = 2 * (c + 1), 2 * (c + 1) + 1
        attn_head(c, 0, 0, 0)
        if c + 1 < C:
            proj_qk(j0, wq_sb, qt_sb, "qk")
        attn_head(c, 0, 1, 1)
        if c + 1 < C:
            proj_qk(j1, wq_sb, qt_sb, "qk")
        if c < C - 1:
            yproj(c, 0)
        if c + 1 < C:
            proj_qk(j0, wk_sb, kt_sb, "qk")
        attn_head(c, 1, 0, 0)
        if c + 1 < C:
            proj_qk(j1, wk_sb, kt_sb, "qk")
        if c == C - 1:
            yproj(c, 0)
        attn_head(c, 1, 1, 1)
        if c + 1 < C:
            proj_v(j0, "qk")
        yproj(c, 1)
        if c + 1 < C:
            proj_v(j1, "qk")


_CACHE = {}


def _build():
    if "nc" in _CACHE:
        return _CACHE["nc"]
    nc = bacc.Bacc(
        "TRN2",
        target_bir_lowering=False,
        debug=False,
        enable_asserts=False,
        num_devices=NCORES,
    )
    ins = {}
    for i in range(NDT):
        ins[f"xt{i}"] = nc.dram_tensor(f"xt{i}", [128, T], BF16,
                                       kind="ExternalInput").ap()
    for nm in ("wq", "wk", "wv"):
        ins[nm] = nc.dram_tensor(nm, [D, HDB], BF16, kind="ExternalInput").ap()
    ins["wo"] = nc.dram_tensor("wo", [HDB, D], BF16, kind="ExternalInput").ap()
    ins["ident"] = nc.dram_tensor("ident", [128, 128], BF16,
                                  kind="ExternalInput").ap()
    outs = {"y": nc.dram_tensor("y", [T, D], BF16, kind="ExternalOutput").ap()}
    with tile.TileContext(nc, trace_sim=False) as tc:
        with ExitStack() as kctx:
            build_kernel(nc, tc, outs, ins, kctx)
    nc.compile()
    _CACHE["nc"] = nc
    return nc


def make_in_maps(x, Wq, Wk, Wv, Wo, bo):
    xv = np.asarray(x, np.float32).reshape(T, D).T          # [D, T]
    x16 = np.ascontiguousarray(xv).astype(ml_dtypes.bfloat16)
    ident = np.eye(128, dtype=np.float32).astype(ml_dtypes.bfloat16)
    in_maps = []
    for core in range(NCORES):
        hs = slice(core * HDB, (core + 1) * HDB)
        m = {f"xt{i}": x16[i * 128:(i + 1) * 128] for i in range(NDT)}
        m["wq"] = np.ascontiguousarray(
            np.asarray(Wq, np.float32)[:, hs]).astype(ml_dtypes.bfloat16)
        m["wk"] = np.ascontiguousarray(
            np.asarray(Wk, np.float32)[:, hs]).astype(ml_dtypes.bfloat16)
        m["wv"] = np.ascontiguousarray(
            np.asarray(Wv, np.float32)[:, hs]).astype(ml_dtypes.bfloat16)
        m["wo"] = np.ascontiguousarray(
            np.asarray(Wo, np.float32)[hs, :]).astype(ml_dtypes.bfloat16)
        m["ident"] = ident
        in_maps.append(m)
    return in_maps


def kernel(x, Wq, Wk, Wv, Wo, bo, _trace=False, _tmpdir=None):
    nc = _build()
    in_maps = make_in_maps(x, Wq, Wk, Wv, Wo, bo)
    res = run_bass_kernel_spmd(
        nc, in_maps, core_ids=list(range(NCORES)),
        trace=_trace, tmpdir=_tmpdir,
        **({"trace_cores": list(range(NCORES))} if _trace else {}),
    )
    if _trace:
        kernel.last_results = res
    y = np.zeros((T, D), dtype=np.float32)
    for r in res.results:
        y += np.asarray(r["y"], dtype=np.float32)
    y += np.asarray(bo, np.float32).reshape(1, D)
    return y.reshape(B, F, N, D)


# revision 12
# speedup vs baseline: 1.4414x; 1.0593x over previous
"""LocalFrameAttentionWithDiffuser on 8 TRN2 NeuronCores.

Sharding: head-parallel. Each core computes 2 of the 16 heads end-to-end
(QKV projection for its 128 hd-dims, chunked local attention, partial
output projection Y_c = O_c @ Wo[c-slice]); the host sums the 8 partial
Y tensors in fp32 and adds the bias once.

Shapes (hardcoded from the problem):
  x [1,16,256,1024] -> tokens T=4096, D=1024, H=16 heads, HD=64,
  chunks C=4 of L=1024 tokens; chunk i attends to chunks {i-1, i}
  (chunk 0 only to itself).

Device layout notes:
  - all matmul operands are bf16 (x, weights converted on host; DMA and
    SBUF halve vs fp32; PE rate is the same as fp32r).
  - S^T = K^T.T @ Q^T per (chunk, head) with ctx on partitions; exp runs
    on the scalar engine over [128, 2, 512] PSUM pairs (two ctx tiles
    per instruction to amortize the fixed access latency) and the
    softmax sum comes from a ones-column appended to V in the AV matmul.
  - chunk 0's missing previous chunk is handled by not issuing those ctx
    tiles (exactly reproduces the -inf mask).
  - partial Y is written bf16 straight after the output projection (the
    bias is added host-side, once); the host accumulates in fp32.
  - projections of chunk c+1 are interleaved between the attention heads
    of chunk c so the scalar engine (the bottleneck) never starves.
"""

import os
from contextlib import ExitStack

import numpy as np
import ml_dtypes

import concourse.bass as bass
import concourse.tile as tile
from concourse import bacc, mybir
from concourse.bass_utils import run_bass_kernel_spmd

F32 = mybir.dt.float32
BF16 = mybir.dt.bfloat16
F8 = mybir.dt.float8e4
DR = mybir.MatmulPerfMode.DoubleRow
WS = 64.0

B, F, N, D = 1, 16, 256, 1024
H, HD = 16, 64
CS = 4
C = F // CS            # 4 chunks
L = CS * N             # 1024 tokens per chunk
T = F * N              # 4096 tokens
NCORES = 8
HPC = H // NCORES      # 2 heads per core
HDB = HPC * HD         # 128 hd dims per core
SCALE = 1.0 / np.sqrt(HD)

TOK_TILE = 512
NDT = D // 128         # 8 contraction tiles for projections
NCT = T // 128         # 32 ctx tiles of 128


def build_kernel(nc, tc, outs, ins, ctx):
    xt = [ins[f"xt{i}"] for i in range(NDT)]
    x8 = [ins[f"x8{i}"] for i in range(NDT)]
    wq8, wk8 = ins["wq8"], ins["wk8"]
    wv, wo, ident = ins["wv"], ins["wo"], ins["ident"]
    y = outs["y"]

    wpool = ctx.enter_context(tc.tile_pool(name="weights", bufs=1))
    act_pool = ctx.enter_context(tc.tile_pool(name="acts", bufs=1))
    vs_pool = ctx.enter_context(tc.tile_pool(name="vstage", bufs=2))
    a_pool = ctx.enter_context(tc.tile_pool(name="attn", bufs=4))
    sum_pool = ctx.enter_context(tc.tile_pool(name="sums", bufs=4))
    bc_pool = ctx.enter_context(tc.tile_pool(name="bcast", bufs=4))
    yout_pool = ctx.enter_context(tc.tile_pool(name="yout", bufs=2))
    ps_pool = ctx.enter_context(tc.tile_pool(name="ps", bufs=1, space="PSUM"))

    # ---- persistent weights / activations ----
    wq_sb = wpool.tile([128, NDT // 2, 2, HDB], F8, tag="wq")
    wk_sb = wpool.tile([128, NDT // 2, 2, HDB], F8, tag="wk")
    wv_sb = wpool.tile([128, NDT, HDB], BF16, tag="wv")
    x8_sb = [wpool.tile([128, 2, T], F8, name=f"x8s{i}") for i in range(NDT)]
    wo_sb = wpool.tile([128, D], BF16, tag="wo")
    id_sb = wpool.tile([128, 128], BF16, tag="id")
    xt_sb = [wpool.tile([128, T], BF16, name=f"xs{i}") for i in range(NDT)]

    qt_sb = act_pool.tile([128, T], BF16, tag="qt")   # Q^T (2 heads stacked)
    kt_sb = act_pool.tile([128, T], BF16, tag="kt")   # K^T
    ot_sb = act_pool.tile([128, T], BF16, tag="ot")   # normalized O^T
    # V per head: [128 ctx, ct, 64 hd + ones]
    v_sb = [act_pool.tile([128, NCT, HD + 1], BF16, name=f"v{h}")
            for h in range(HPC)]
    for h in range(HPC):
        nc.vector.memset(v_sb[h][:, :, HD:HD + 1], 1.0)

    def dma_x(c):
        for i in range(NDT):
            nc.sync.dma_start(
                x8_sb[i][:, :, c * L:(c + 1) * L],
                x8[i].rearrange("p (k t) -> p k t", k=2)[:, :, c * L:(c + 1) * L])
        for i in range(NDT):
            nc.sync.dma_start(xt_sb[i][:, c * L:(c + 1) * L],
                              xt[i][:, c * L:(c + 1) * L])

    dma_x(0)
    nc.sync.dma_start(wq_sb[:],
                      wq8.rearrange("(i p) (k m) -> p i k m", i=NDT // 2, k=2))
    nc.sync.dma_start(wk_sb[:],
                      wk8.rearrange("(i p) (k m) -> p i k m", i=NDT // 2, k=2))
    nc.sync.dma_start(wv_sb[:], wv.rearrange("(i p) m -> p i m", i=NDT))
    nc.sync.dma_start(wo_sb[:], wo[:, :])
    nc.sync.dma_start(id_sb[:], ident[:, :])

    def proj_qk(j, w_sb, dst, tag):
        tok = slice(j * TOK_TILE, (j + 1) * TOK_TILE)
        p_ps = ps_pool.tile([128, TOK_TILE], F32, tag=tag, name=f"p{j}_{tag}")
        for i in range(NDT):
            nc.tensor.matmul(p_ps[:], w_sb[:, i % 4, :, :], x8_sb[i][:, :, tok],
                             start=(i == 0), stop=(i == NDT - 1),
                             perf_mode=DR)
        nc.vector.tensor_copy(dst[:, tok], p_ps[:])

    def proj_v(j, tag):
        tok = slice(j * TOK_TILE, (j + 1) * TOK_TILE)
        v_ps = ps_pool.tile([128, TOK_TILE], F32, tag=tag, name=f"vps{j}")
        for i in range(NDT):
            nc.tensor.matmul(v_ps[:], wv_sb[:, i, :], xt_sb[i][:, tok],
                             start=(i == 0), stop=(i == NDT - 1))
        vt_stage = vs_pool.tile([128, TOK_TILE], BF16, tag="vs", name=f"vst{j}")
        nc.vector.tensor_copy(vt_stage[:], v_ps[:])
        for m in range(4):
            ct = j * 4 + m
            vtr = ps_pool.tile([128, 128], BF16, tag="yv", name=f"vtr{j}_{m}")
            nc.tensor.transpose(vtr[:], vt_stage[:, m * 128:(m + 1) * 128],
                                id_sb[:])
            for h in range(HPC):
                nc.vector.tensor_copy(v_sb[h][:, ct, 0:HD],
                                      vtr[:, h * HD:(h + 1) * HD])

    def attn_head(c, th, h, tag):
        """Attention for (chunk c, token half th, head h) -> ot_sb."""
        tok = slice(c * L + th * TOK_TILE, c * L + (th + 1) * TOK_TILE)
        hr = slice(h * HD, (h + 1) * HD)
        ct0 = max(0, 8 * (c - 1))
        cts = list(range(ct0, 8 * (c + 1)))
        o_ps = ps_pool.tile([HD + 1, TOK_TILE], F32, tag=f"o{tag}",
                            name=f"ops{c}_{th}_{h}")
        npair = len(cts) // 2
        for gi in range(npair):
            s2 = ps_pool.tile([128, 2, TOK_TILE], F32, tag="s", bufs=2,
                              name=f"sps{c}_{th}_{h}_{gi}")
            for kk in range(2):
                ct = cts[2 * gi + kk]
                nc.tensor.matmul(s2[:, kk, :],
                                 kt_sb[hr, ct * 128:(ct + 1) * 128],
                                 qt_sb[hr, tok], start=True, stop=True)
            a_t = a_pool.tile([128, 2, TOK_TILE], BF16, tag="a",
                              name=f"a{c}_{th}_{h}_{gi}")
            nc.scalar.activation(a_t[:], s2[:],
                                 mybir.ActivationFunctionType.Exp,
                                 scale=float(SCALE / (WS * WS)))
            for kk in range(2):
                ct = cts[2 * gi + kk]
                nc.tensor.matmul(o_ps[:], v_sb[h][:, ct, :], a_t[:, kk, :],
                                 start=(gi == 0 and kk == 0),
                                 stop=(gi == npair - 1 and kk == 1))
        s_sum = sum_pool.tile([1, TOK_TILE], F32, tag="ss", name=f"ssum{c}_{th}_{h}")
        nc.vector.reciprocal(s_sum[:], o_ps[HD:HD + 1, :])
        r_bc = bc_pool.tile([HD, TOK_TILE], F32, tag="bc", name=f"bc{c}_{th}_{h}")
        nc.gpsimd.partition_broadcast(r_bc[:], s_sum[0:1, :])
        nc.vector.tensor_mul(ot_sb[hr, tok], o_ps[0:HD, :], r_bc[:])

    def yproj(c, th):
        """Output projection for the 4 token tiles of half-chunk (c, th)."""
        m0 = c * 8 + th * 4
        for dh in range(2):
            ds = slice(dh * TOK_TILE, (dh + 1) * TOK_TILE)
            y_sb = yout_pool.tile([128, 4, TOK_TILE], BF16, tag="yo",
                                  name=f"yo{c}_{th}_{dh}")
            for mi in range(4):
                m = m0 + mi
                y_ps = ps_pool.tile([128, TOK_TILE], F32, tag="yv",
                                    name=f"yps{m}_{dh}")
                nc.tensor.matmul(y_ps[:], ot_sb[:, m * 128:(m + 1) * 128],
                                 wo_sb[:, ds], start=True, stop=True)
                nc.vector.tensor_copy(y_sb[:, mi, :], y_ps[:])
            nc.sync.dma_start(
                y[m0 * 128:(m0 + 4) * 128, ds]
                .rearrange("(m p) t -> p m t", m=4), y_sb[:])

    # ---- schedule ----
    # prologue: chunk-0 projections, pipelined through 3 psum slots
    for j, tg in ((0, "qk"), (1, "o0")):
        proj_qk(j, wq_sb, qt_sb, tg)
    for j, tg in ((0, "o1"), (1, "qk")):
        proj_qk(j, wk_sb, kt_sb, tg)
    proj_v(0, "o0")
    proj_v(1, "o1")
    for c in range(C):
        if c + 1 < C:
            dma_x(c + 1)
        j0, j1 = 2 * (c + 1), 2 * (c + 1) + 1
        attn_head(c, 0, 0, 0)
        if c + 1 < C:
            proj_qk(j0, wq_sb, qt_sb, "qk")
        attn_head(c, 0, 1, 1)
        if c + 1 < C:
            proj_qk(j1, wq_sb, qt_sb, "qk")
        if c < C - 1:
            yproj(c, 0)
        if c + 1 < C:
            proj_qk(j0, wk_sb, kt_sb, "qk")
        attn_head(c, 1, 0, 0)
        if c + 1 < C:
            proj_qk(j1, wk_sb, kt_sb, "qk")
        if c == C - 1:
            yproj(c, 0)
        attn_head(c, 1, 1, 1)
        if c + 1 < C:
            proj_v(j0, "qk")
        yproj(c, 1)
        if c + 1 < C:
            proj_v(j1, "qk")


_CACHE = {}


def _build():
    if "nc" in _CACHE:
        return _CACHE["nc"]
    nc = bacc.Bacc(
        "TRN2",
        target_bir_lowering=False,
        debug=False,
        enable_asserts=False,
        num_devices=NCORES,
    )
    ins = {}
    for i in range(NDT):
        ins[f"xt{i}"] = nc.dram_tensor(f"xt{i}", [128, T], BF16,
                                       kind="ExternalInput").ap()
    for i in range(NDT):
        ins[f"x8{i}"] = nc.dram_tensor(f"x8{i}", [128, 2 * T], F8,
                                       kind="ExternalInput").ap()
    for nm in ("wq8", "wk8"):
        ins[nm] = nc.dram_tensor(nm, [D // 2, 2 * HDB], F8,
                                 kind="ExternalInput").ap()
    ins["wv"] = nc.dram_tensor("wv", [D, HDB], BF16, kind="ExternalInput").ap()
    ins["wo"] = nc.dram_tensor("wo", [HDB, D], BF16, kind="ExternalInput").ap()
    ins["ident"] = nc.dram_tensor("ident", [128, 128], BF16,
                                  kind="ExternalInput").ap()
    outs = {"y": nc.dram_tensor("y", [T, D], BF16, kind="ExternalOutput").ap()}
    with tile.TileContext(nc, trace_sim=False) as tc:
        with ExitStack() as kctx:
            build_kernel(nc, tc, outs, ins, kctx)
    nc.compile()
    _CACHE["nc"] = nc
    return nc


def make_in_maps(x, Wq, Wk, Wv, Wo, bo):
    xv = np.asarray(x, np.float32).reshape(T, D).T          # [D, T]
    x16 = np.ascontiguousarray(xv).astype(ml_dtypes.bfloat16)
    ident = np.eye(128, dtype=np.float32).astype(ml_dtypes.bfloat16)
    xhi = xv.astype(ml_dtypes.float8_e4m3)
    xlo = (xv - xhi.astype(np.float32)).astype(ml_dtypes.float8_e4m3)
    x8t = np.concatenate([
        np.ascontiguousarray(
            xx.astype(np.float32).reshape(NDT // 2, 2, 128, T)
            .transpose(0, 2, 1, 3).reshape(NDT // 2, 128, 2 * T))
        .astype(ml_dtypes.float8_e4m3)
        for xx in (xhi, xlo)], axis=0)

    def w8prep(W, hs):
        w = np.asarray(W, np.float32)[:, hs] * WS
        w = w.reshape(NDT // 2, 2, 128, HDB).transpose(0, 2, 1, 3)
        return np.ascontiguousarray(
            w.reshape(D // 2, 2 * HDB)).astype(ml_dtypes.float8_e4m3)

    in_maps = []
    for core in range(NCORES):
        hs = slice(core * HDB, (core + 1) * HDB)
        m = {f"xt{i}": x16[i * 128:(i + 1) * 128] for i in range(NDT)}
        for i in range(NDT):
            m[f"x8{i}"] = x8t[i]
        m["wq8"] = w8prep(Wq, hs)
        m["wk8"] = w8prep(Wk, hs)
        m["wv"] = np.ascontiguousarray(
            np.asarray(Wv, np.float32)[:, hs]).astype(ml_dtypes.bfloat16)
        m["wo"] = np.ascontiguousarray(
            np.asarray(Wo, np.float32)[hs, :]).astype(ml_dtypes.bfloat16)
        m["ident"] = ident
        in_maps.append(m)
    return in_maps


def kernel(x, Wq, Wk, Wv, Wo, bo, _trace=False, _tmpdir=None):
    nc = _build()
    in_maps = make_in_maps(x, Wq, Wk, Wv, Wo, bo)
    res = run_bass_kernel_spmd(
        nc, in_maps, core_ids=list(range(NCORES)),
        trace=_trace, tmpdir=_tmpdir,
        **({"trace_cores": list(range(NCORES))} if _trace else {}),
    )
    if _trace:
        kernel.last_results = res
    y = np.zeros((T, D), dtype=np.float32)
    for r in res.results:
        y += np.asarray(r["y"], dtype=np.float32)
    y += np.asarray(bo, np.float32).reshape(1, D)
    return y.reshape(B, F, N, D)
